# revision 31
# baseline (speedup 1.0000x reference)
"""AttentionSequencePoolingLayer (DIN) — Bass/Tile Trainium2 kernel.

Contract: kernel(**inputs) takes FULL unsharded inputs
(query_ad [1024,1,64] f32, user_behavior [1024,200,64] f32,
user_behavior_length [1024,1] int64, W1/b1/a1, W2/b2/a2, W3/b3/a3)
and returns the FULL output [1024, 64] f32.

Strategy: pure data-parallel over batch (128 rows per core, 8 cores, no
collectives).  Per-core math uses per-shard dice batch-norm statistics
(explicitly allowed by the sharding hint; n>=12800 samples per stat).

Activations and weights are bf16 on chip (matmuls accumulate fp32 in
PSUM; statistics, softmax scalars and the final output are fp32), which
runs the PE at 1 cycle/row, enables the DVE 4x mode, and halves HBM
traffic.  user_behavior is pre-converted to bf16 on the host.

On-chip layout ("t-parity packed transposed"): a core's tokens map to
columns n = tp*128 + b (tp = t//2), partitions hold (par, channel) with
par = t % 2.  The packed transpose of user_behavior is produced by
DMA-transpose straight from DRAM (one [128,128] xbar transpose per
t-pair) — no PE transposes or PSUM round-trip.  Matmuls use
block-diagonal weights blockdiag(W, W) so one K=128 matmul serves both
parity halves per 512-column chunk.

Algebra: att_in @ W1 decomposes as q@(W1a+W1c) + ub@(W1b-W1c) + (q*ub)@W1d.
The per-(b,c) term qW = q@Wq + b1 is accumulated into layer-1 PSUM via a
third matmul against a constant selector matrix (I_128 tiled 4x), so the
PSUM->SBUF copy is a plain copy.
"""

import os
import sys
from contextlib import ExitStack

import numpy as np

sys.path.insert(0, "/opt/trn_rl_repo")

import concourse.bacc as bacc
import concourse.bass as bass
import concourse.bass_isa as bass_isa
import concourse.library_config as library_config
import concourse.tile as tile
from concourse import mybir
from concourse.bass_utils import run_bass_kernel_spmd

F32 = mybir.dt.float32
BF16 = mybir.dt.bfloat16
I32 = mybir.dt.int32

N_CORES = 8
B, T, E = 1024, 200, 64
H1, H2 = 64, 16
Bs = B // N_CORES          # 128 batch rows per core
TP = T // 2                # 100 t-pairs
NCOL = TP * Bs             # 12800 packed columns per core
NCHUNK = NCOL // 512       # 25 column chunks of 512
EPS = 1e-8

# Module-global: test.py reads this after calling kernel() for trace info.
LAST_RESULTS = None


def _install_ntff_hook():
    """The agent image's antenv lacks axon_hooks, so trn_boot's NTFF hook
    registration silently degrades.  Recreate the registry module and set
    the ctypes-based hook so run_bass_kernel_spmd(trace=True) captures
    NTFF profiles."""
    import sys as _sys
    import types as _types

    try:
        from antenv.axon_hooks import get_axon_ntff_profile_hook  # noqa
        return
    except ImportError:
        pass
    try:
        from trn_agent_boot.trn_boot import _ntff_profile_via_ctypes
        hook = _ntff_profile_via_ctypes("/opt/axon/libaxon_pjrt.so")
        mod = _types.ModuleType("antenv.axon_hooks")
        _slot = [hook]
        mod.set_axon_ntff_profile_hook = lambda h: _slot.__setitem__(0, h)
        mod.get_axon_ntff_profile_hook = lambda: _slot[0]
        _sys.modules["antenv.axon_hooks"] = mod
        import antenv
        antenv.axon_hooks = mod
    except Exception as e:  # degrade silently — trace is optional
        print(f"ntff hook install failed: {e}")


def _ap(t, offset_elems, pattern):
    """Raw AP view over a tile AP (0-stride broadcasts, strided views).

    AP offsets are in elements; `pattern` dims are [elem_step, count] with
    dim0 = partition dim (pass through t.ap[0])."""
    return bass.AP(tensor=t.tensor, offset=t.offset + offset_elems, ap=pattern)


def _build(b3f, alphas_zero):
    nc = bacc.Bacc(
        "TRN2", target_bir_lowering=False, debug=False, num_devices=N_CORES
    )

    def din(name, shape, dt):
        return nc.dram_tensor(name, list(shape), dt, kind="ExternalInput").ap()

    ub_d = din("ub", [Bs, T, E], BF16)
    q_d = din("q", [Bs, E], BF16)
    lens_d = din("lens", [Bs, 1], I32)
    eye_d = din("eye", [32, 32], BF16)
    wublk_d = din("wublk", [128, 128], BF16)   # blockdiag(Wu, Wu)
    wpblk_d = din("wpblk", [128, 128], BF16)   # blockdiag(Wp, Wp)
    w2blk_d = din("w2blk", [128, 2 * H2], BF16)  # blockdiag(W2, W2)
    wq_d = din("wq", [E, H1], BF16)
    w3rep_d = din("w3rep", [128, 2 * H2], BF16)  # tile(W3, (128, 2))
    b1row_d = din("b1row", [128, H1], F32)     # b1 replicated down partitions
    b2dup_d = din("b2dup", [1, 2 * H2], BF16)
    sel_d = din("sel", [128, 512], BF16)       # I_128 tiled 4x
    iota_d = din("iota", [128, T], F32)
    if not alphas_zero:
        a1r_d = din("a1r", [128, 1], F32)
        am1r_d = din("am1r", [128, 1], F32)
        a2r_d = din("a2r", [2 * H2, 1], F32)
        am2r_d = din("am2r", [2 * H2, 1], F32)
    out_d = nc.dram_tensor("out", [Bs, E], F32, kind="ExternalOutput").ap()

    with tile.TileContext(nc) as tc, ExitStack() as ctx:
        # ---- pools ---------------------------------------------------------
        consts = ctx.enter_context(tc.tile_pool(name="consts", bufs=1))
        smalls = ctx.enter_context(tc.tile_pool(name="smalls", bufs=1))
        big = ctx.enter_context(tc.tile_pool(name="big", bufs=1))
        whp = ctx.enter_context(tc.tile_pool(name="whp", bufs=2))
        ph1 = ctx.enter_context(tc.tile_pool(name="ph1", bufs=3, space="PSUM"))
        ph2 = ctx.enter_context(tc.tile_pool(name="ph2", bufs=2, space="PSUM"))
        pt3 = ctx.enter_context(tc.tile_pool(name="pt3", bufs=2, space="PSUM"))

        # ---- packed DMA-transpose of ub, FIRST -----------------------------
        # The XPOSE descriptor struct only has one sync-wait slot, so every
        # other DMA is forced to schedule after the transposes (order-only
        # deps): cross-queue FIFO waits then land on copy-mode DMAs, which
        # have enough wait slots.
        ubT = big.tile([128, NCOL], BF16, tag="ubT")
        ubT3 = ubT.rearrange("p (k b) -> p k b", b=128)
        for i in range(NCHUNK):
            # [128 b, 8 t, 64 e] -> per t-pair k: out[(t%2,e), k, b]
            nc.sync.dma_start_transpose(
                out=ubT3[:, 4 * i : 4 * (i + 1), :],
                in_=ub_d[:, 8 * i : 8 * i + 8, :],
            )

        dma = nc.gpsimd.dma_start

        # ---- constants -----------------------------------------------------
        def cin(name, dram, shape, dt):
            t = consts.tile(shape, dt, name=name)
            dma(out=t, in_=dram)
            return t

        eye = cin("eye", eye_d, [32, 32], BF16)
        wublk = cin("wublk", wublk_d, [128, 128], BF16)
        wpblk = cin("wpblk", wpblk_d, [128, 128], BF16)
        w2blk = cin("w2blk", w2blk_d, [128, 2 * H2], BF16)
        wq = cin("wq", wq_d, [E, H1], BF16)
        w3rep = cin("w3rep", w3rep_d, [128, 2 * H2], BF16)
        b1row = cin("b1row", b1row_d, [128, H1], F32)
        b2dup = cin("b2dup", b2dup_d, [1, 2 * H2], BF16)
        sel = cin("sel", sel_d, [128, 512], BF16)
        iota = cin("iota", iota_d, [128, T], F32)
        lens_i = cin("lens_i", lens_d, [Bs, 1], I32)
        if not alphas_zero:
            a1r = cin("a1r", a1r_d, [128, 1], F32)
            am1r = cin("am1r", am1r_d, [128, 1], F32)
            a2r = cin("a2r", a2r_d, [2 * H2, 1], F32)
            am2r = cin("am2r", am2r_d, [2 * H2, 1], F32)

        lens_f = smalls.tile([Bs, 1], F32)
        nc.vector.tensor_copy(out=lens_f, in_=lens_i)
        eps_t = smalls.tile([128, 1], F32)
        nc.vector.memset(eps_t, EPS)

        # ---- q^T (DMA transpose), qW = q@Wq + b1 dup'd along free ----------
        qT = consts.tile([128, 128], BF16)
        dma(out=qT[0:E, :], in_=q_d.rearrange("a b -> b a"))
        dma(out=qT[E : 2 * E, :], in_=qT[0:E, :])

        pqw = ph1.tile([Bs, H1], F32, tag="pqw", bufs=1)
        nc.tensor.matmul(pqw, lhsT=qT[0:E, :], rhs=wq, start=True, stop=True)
        qWdup = consts.tile([128, 128], BF16)
        nc.vector.tensor_add(out=qWdup[:, 0:H1], in0=pqw, in1=b1row)
        nc.scalar.copy(out=qWdup[:, H1:128], in_=qWdup[:, 0:H1])

        # ---- big SBUF buffers (slots reused across phases via tags) --------
        qub = big.tile([128, NCOL], BF16, tag="qub")
        h1 = big.tile([128, NCOL], BF16, tag="h1")
        st1 = smalls.tile([128, NCHUNK, 6], F32)
        st2 = smalls.tile([32, NCHUNK, 6], F32)

        # ---- phase 1: qub -> L1 -> h1 --------------------------------------
        for i in range(NCHUNK):
            cols = slice(512 * i, 512 * (i + 1))
            # qub = ubT * q (q broadcast along the 4 tp's of the chunk)
            qbc = _ap(qT, 0, [qT.ap[0], [0, 4], [1, 128]])
            nc.vector.tensor_mul(out=qub[:, cols], in0=ubT[:, cols], in1=qbc)
            # L1: ub-term + qub-term + qW-via-selector, accumulated in PSUM
            p1t = ph1.tile([128, 512], F32, tag="h1p")
            nc.tensor.matmul(p1t, lhsT=wublk, rhs=ubT[:, cols],
                             start=True, stop=False)
            nc.tensor.matmul(p1t, lhsT=wpblk, rhs=qub[:, cols],
                             start=False, stop=False)
            nc.tensor.matmul(p1t, lhsT=qWdup, rhs=sel,
                             start=False, stop=True)
            if i % 2 == 0:
                nc.scalar.copy(out=h1[:, cols], in_=p1t)
            else:
                nc.vector.tensor_copy(out=h1[:, cols], in_=p1t)
            nc.vector.bn_stats(out=st1[:, i, :], in_=h1[:, cols])

        # ---- dice1 ---------------------------------------------------------
        def dice_stats(st, parts, key):
            mv = smalls.tile([parts, 2], F32, name=f"mv_{key}")
            nc.vector.bn_aggr(out=mv, in_=st)
            rs = smalls.tile([parts, 1], F32, name=f"rs_{key}")
            nc.scalar.activation(
                out=rs, in_=mv[:, 1:2], func=mybir.ActivationFunctionType.Sqrt,
                bias=eps_t[0:parts], scale=1.0,
            )
            nc.vector.reciprocal(out=rs, in_=rs)
            nb = smalls.tile([parts, 1], F32, name=f"nb_{key}")
            nc.vector.tensor_scalar(
                out=nb, in0=mv[:, 0:1], scalar1=rs, scalar2=-1.0,
                op0=mybir.AluOpType.mult, op1=mybir.AluOpType.mult,
            )
            return rs, nb

        rs1, nb1 = dice_stats(st1, 128, "d1")
        p1s = big.tile([128, NCOL], BF16, tag="qub")  # reuse qub slot
        for j in range(4):
            cols = slice(3200 * j, 3200 * (j + 1))
            nc.scalar.activation(
                out=p1s[:, cols], in_=h1[:, cols],
                func=mybir.ActivationFunctionType.Sigmoid, bias=nb1, scale=rs1,
            )
            if not alphas_zero:
                nc.vector.tensor_scalar(
                    out=p1s[:, cols], in0=p1s[:, cols], scalar1=am1r, scalar2=a1r,
                    op0=mybir.AluOpType.mult, op1=mybir.AluOpType.add,
                )
            nc.vector.tensor_mul(out=h1[:, cols], in0=h1[:, cols], in1=p1s[:, cols])

        # ---- L2: one block-diagonal matmul per chunk -> h2 [32, 12800] -----
        # b2 is accumulated in PSUM via a K=1 rank-1 matmul (b2 ⊗ ones) so
        # the PSUM->SBUF copy stays a plain table-free Copy.
        ones_row = smalls.tile([1, 512], BF16)
        nc.vector.memset(ones_row, 1.0)
        h2 = big.tile([32, NCOL], BF16, tag="ubT")  # reuse ubT slot
        for i in range(NCHUNK):
            cols = slice(512 * i, 512 * (i + 1))
            p2t = ph2.tile([32, 512], F32, tag="h2p")
            nc.tensor.matmul(p2t, lhsT=w2blk, rhs=h1[:, cols],
                             start=True, stop=False)
            nc.tensor.matmul(p2t, lhsT=b2dup, rhs=ones_row,
                             start=False, stop=True)
            if i % 2 == 0:
                nc.scalar.copy(out=h2[:, cols], in_=p2t)
            else:
                nc.vector.tensor_copy(out=h2[:, cols], in_=p2t)
            nc.vector.bn_stats(out=st2[:, i, :], in_=h2[:, cols])

        # ---- dice2 ---------------------------------------------------------
        rs2, nb2 = dice_stats(st2, 32, "d2")
        p2s = big.tile([32, NCOL], BF16, tag="qub")  # reuse qub slot again
        for j in range(4):
            cols = slice(3200 * j, 3200 * (j + 1))
            nc.scalar.activation(
                out=p2s[:, cols], in_=h2[:, cols],
                func=mybir.ActivationFunctionType.Sigmoid, bias=nb2, scale=rs2,
            )
            if not alphas_zero:
                nc.vector.tensor_scalar(
                    out=p2s[:, cols], in0=p2s[:, cols], scalar1=am2r, scalar2=a2r,
                    op0=mybir.AluOpType.mult, op1=mybir.AluOpType.add,
                )
            nc.vector.tensor_mul(out=h2[:, cols], in0=h2[:, cols], in1=p2s[:, cols])

        # ---- L3: transpose h2 back to batch-major, dot with W3 on DVE ------
        # Groups of 16 column-subblocks of 128: transpose [32,128] -> [128,32]
        sc = smalls.tile([Bs, T], F32)
        NG = (TP + 15) // 16  # 7 groups (6 full, last has 4 subblocks)
        for g in range(NG):
            nsub = min(16, TP - 16 * g)
            p3t = pt3.tile([128, 512], BF16, tag="t3")
            for s in range(nsub):
                k = 16 * g + s
                nc.tensor.matmul(
                    p3t[:, 32 * s : 32 * (s + 1)],
                    lhsT=h2[:, 128 * k : 128 * (k + 1)],
                    rhs=eye, is_transpose=True,
                )
            whj = whp.tile([128, 512], BF16, tag="wh2")
            w3b = _ap(w3rep, 0, [w3rep.ap[0], [0, nsub], [1, 32]])
            nc.vector.tensor_mul(
                out=whj[:, 0 : 32 * nsub], in0=p3t[:, 0 : 32 * nsub], in1=w3b
            )
            red_in = _ap(whj, 0, [whj.ap[0], [32, nsub], [16, 2], [1, 16]])
            red_out = _ap(sc, 32 * g, [sc.ap[0], [2, nsub], [1, 2]])
            nc.vector.tensor_reduce(
                out=red_out, in_=red_in, axis=mybir.AxisListType.X,
                op=mybir.AluOpType.add,
            )

        # ---- dice3 (per-core global stats via PE ones-matmul reduce) -------
        st3 = smalls.tile([128, 2], F32)
        nc.vector.reduce_sum(out=st3[:, 0:1], in_=sc, axis=mybir.AxisListType.X)
        sq_scr = smalls.tile([Bs, T], F32)
        nc.scalar.square(out=sq_scr, in_=sc)
        nc.vector.reduce_sum(out=st3[:, 1:2], in_=sq_scr,
                             axis=mybir.AxisListType.X)
        ones = smalls.tile([128, 1], F32)
        nc.vector.memset(ones, 1.0)
        ptot = ph1.tile([1, 2], F32, tag="pqw", bufs=1)
        nc.tensor.matmul(ptot, lhsT=ones, rhs=st3, start=True, stop=True)
        tot = smalls.tile([1, 2], F32)
        nc.vector.tensor_copy(out=tot, in_=ptot)
        ones_r = smalls.tile([1, 128], F32)
        nc.vector.memset(ones_r, 1.0)
        pbc = ph2.tile([128, 2], F32, tag="h2p")
        nc.tensor.matmul(pbc, lhsT=ones_r, rhs=tot, start=True, stop=True)
        st3r = smalls.tile([128, 2], F32)
        nc.vector.tensor_copy(out=st3r, in_=pbc)
        n_tok = float(Bs * T)
        m3 = smalls.tile([128, 1], F32)
        nc.vector.tensor_scalar(
            out=m3, in0=st3r[:, 0:1], scalar1=1.0 / n_tok, scalar2=None,
            op0=mybir.AluOpType.mult,
        )
        v3 = smalls.tile([128, 1], F32)
        nc.vector.tensor_scalar(
            out=v3, in0=st3r[:, 1:2], scalar1=1.0 / n_tok, scalar2=None,
            op0=mybir.AluOpType.mult,
        )
        msq = smalls.tile([128, 1], F32)
        nc.vector.tensor_mul(out=msq, in0=m3, in1=m3)
        nc.vector.tensor_sub(out=v3, in0=v3, in1=msq)
        rs3 = smalls.tile([128, 1], F32)
        nc.scalar.activation(
            out=rs3, in_=v3, func=mybir.ActivationFunctionType.Sqrt,
            bias=eps_t, scale=1.0,
        )
        nc.vector.reciprocal(out=rs3, in_=rs3)
        nb3 = smalls.tile([128, 1], F32)
        nc.vector.tensor_scalar(
            out=nb3, in0=m3, scalar1=b3f, scalar2=-1.0,
            op0=mybir.AluOpType.subtract, op1=mybir.AluOpType.mult,
        )
        nc.vector.tensor_mul(out=nb3, in0=nb3, in1=rs3)

        p3s = smalls.tile([Bs, T], F32)
        nc.scalar.activation(
            out=p3s, in_=sc, func=mybir.ActivationFunctionType.Sigmoid,
            bias=nb3, scale=rs3,
        )
        scd = smalls.tile([Bs, T], F32)
        if b3f != 0.0:
            scb = smalls.tile([Bs, T], F32)
            nc.scalar.add(out=scb, in_=sc, add=b3f)
            nc.vector.tensor_mul(out=scd, in0=p3s, in1=scb)
        else:
            nc.vector.tensor_mul(out=scd, in0=p3s, in1=sc)

        # ---- mask + softmax ------------------------------------------------
        pen = smalls.tile([Bs, T], F32)
        nc.vector.tensor_scalar(
            out=pen, in0=iota, scalar1=lens_f, scalar2=-1e30,
            op0=mybir.AluOpType.is_ge, op1=mybir.AluOpType.mult,
        )
        nc.vector.tensor_add(out=scd, in0=scd, in1=pen)
        nmx = smalls.tile([Bs, 1], F32)
        nc.vector.tensor_reduce(
            out=nmx, in_=scd, axis=mybir.AxisListType.X,
            op=mybir.AluOpType.max, negate=True,
        )
        wsm = smalls.tile([Bs, T], BF16)
        nc.scalar.activation(
            out=wsm, in_=scd, func=mybir.ActivationFunctionType.Exp,
            bias=nmx, scale=1.0,
        )
        ssum = smalls.tile([Bs, 1], F32)
        nc.vector.reduce_sum(out=ssum, in_=wsm, axis=mybir.AxisListType.X)
        rcp = smalls.tile([Bs, 1], F32)
        nc.vector.reciprocal(out=rcp, in_=ssum)

        # ---- weighted pooling over user_behavior ---------------------------
        # wub = ub * w (bf16, 4x DVE mode), then a contiguous binary
        # reduction tree over t (200 -> 100 -> 50 -> 25 -> strided finish).
        ub2 = big.tile([Bs, T, E], BF16, tag="ubT")  # reuse slot again
        for c in range(4):
            dma(out=ub2[:, 50 * c : 50 * (c + 1), :],
                in_=ub_d[:, 50 * c : 50 * (c + 1), :])
        wub = big.tile([Bs, T, E], BF16, tag="h1x")
        for c in range(4):
            tsl = slice(50 * c, 50 * (c + 1))
            wbc = _ap(wsm, 50 * c, [wsm.ap[0], [1, 50], [0, E]])
            nc.vector.tensor_mul(out=wub[:, tsl, :], in0=ub2[:, tsl, :], in1=wbc)
        tree = big.tile([Bs, 100, E], F32, tag="tree")
        nc.vector.tensor_add(out=tree, in0=wub[:, 0:100, :], in1=wub[:, 100:200, :])
        nc.vector.tensor_add(out=tree[:, 0:50, :], in0=tree[:, 0:50, :],
                             in1=tree[:, 50:100, :])
        nc.vector.tensor_add(out=tree[:, 0:25, :], in0=tree[:, 0:25, :],
                             in1=tree[:, 25:50, :])
        total = smalls.tile([Bs, E], F32)
        red_in = _ap(tree, 0, [tree.ap[0], [1, E], [E, 25]])
        nc.vector.tensor_reduce(
            out=total, in_=red_in, axis=mybir.AxisListType.X,
            op=mybir.AluOpType.add,
        )
        outt = smalls.tile([Bs, E], F32)
        nc.vector.tensor_scalar_mul(out=outt, in0=total, scalar1=rcp)
        dma(out=out_d, in_=outt)

    nc.compile()
    return nc


def _host_prep(query_ad, user_behavior, user_behavior_length,
               W1, b1, a1, W2, b2, a2, W3, b3, a3):
    import ml_dtypes
    f32 = np.float32
    bf16 = ml_dtypes.bfloat16
    q = np.ascontiguousarray(query_ad[:, 0, :]).astype(bf16)
    ub = np.ascontiguousarray(user_behavior).astype(bf16)
    lens = np.ascontiguousarray(user_behavior_length)[:, 0].astype(np.int32)
    W1 = np.asarray(W1, f32)
    Wq = W1[:E] + W1[2 * E : 3 * E]
    Wu = W1[E : 2 * E] - W1[2 * E : 3 * E]
    Wp = W1[3 * E :]
    W2 = np.asarray(W2, f32)
    W3 = np.asarray(W3, f32).reshape(-1)

    def blkdiag(w):
        k, m = w.shape
        out = np.zeros((2 * k, 2 * m), f32)
        out[:k, :m] = w
        out[k:, m:] = w
        return out.astype(bf16)

    consts = dict(
        eye=np.eye(32, dtype=f32).astype(bf16),
        wublk=blkdiag(Wu),
        wpblk=blkdiag(Wp),
        w2blk=blkdiag(W2),
        wq=np.ascontiguousarray(Wq).astype(bf16),
        w3rep=np.tile(W3, (128, 2)).astype(bf16),
        b1row=np.tile(np.asarray(b1, f32).reshape(1, H1), (128, 1)),
        b2dup=np.tile(np.asarray(b2, f32), 2).reshape(1, 2 * H2).astype(bf16),
        sel=np.tile(np.eye(128, dtype=f32), (1, 4)).astype(bf16),
        iota=np.broadcast_to(np.arange(T, dtype=f32), (128, T)).copy(),
    )
    b3f = float(np.asarray(b3, f32).reshape(-1)[0])
    a1 = np.asarray(a1, f32)
    a2 = np.asarray(a2, f32)
    a3f = float(np.asarray(a3, f32).reshape(-1)[0])
    alphas_zero = not a1.any() and not a2.any() and a3f == 0.0
    if not alphas_zero:
        if a3f != 0.0:
            raise NotImplementedError("nonzero alpha3 not supported")
        a1r = a1[(np.arange(128) % H1)].reshape(128, 1).astype(f32)
        a2r = np.tile(a2, 2).reshape(2 * H2, 1).astype(f32)
        consts.update(a1r=a1r, am1r=1.0 - a1r, a2r=a2r, am2r=1.0 - a2r)
    return q, ub, lens, consts, b3f, alphas_zero


def kernel(query_ad, user_behavior, user_behavior_length,
           W1, b1, a1, W2, b2, a2, W3, b3, a3):
    global LAST_RESULTS
    q, ub, lens, consts, b3f, alphas_zero = _host_prep(
        query_ad, user_behavior, user_behavior_length,
        W1, b1, a1, W2, b2, a2, W3, b3, a3)

    nc = _build(b3f, alphas_zero)

    in_maps = []
    for c in range(N_CORES):
        sl = slice(c * Bs, (c + 1) * Bs)
        m = dict(consts)
        m["ub"] = np.ascontiguousarray(ub[sl])
        m["q"] = np.ascontiguousarray(q[sl])
        m["lens"] = np.ascontiguousarray(lens[sl]).reshape(Bs, 1)
        in_maps.append(m)

    trace = bool(os.environ.get("BASS_TRACE"))
    if trace:
        _install_ntff_hook()
    res = run_bass_kernel_spmd(
        nc, in_maps, core_ids=list(range(N_CORES)), trace=trace,
    )
    LAST_RESULTS = res
    out = np.concatenate([res.results[c]["out"] for c in range(N_CORES)], axis=0)
    return out.astype(np.float32)


# revision 33
# speedup vs baseline: 1.2558x; 1.2558x over previous
"""AttentionSequencePoolingLayer (DIN) — Bass/Tile Trainium2 kernel.

Contract: kernel(**inputs) takes FULL unsharded inputs
(query_ad [1024,1,64] f32, user_behavior [1024,200,64] f32,
user_behavior_length [1024,1] int64, W1/b1/a1, W2/b2/a2, W3/b3/a3)
and returns the FULL output [1024, 64] f32.

Strategy: pure data-parallel over batch (128 rows per core, 8 cores, no
collectives).  Per-core math uses per-shard dice batch-norm statistics
(explicitly allowed by the sharding hint; n>=12800 samples per stat).

Activations and weights are bf16 on chip (matmuls accumulate fp32 in
PSUM; statistics, softmax scalars and the final output are fp32), which
runs the PE at 1 cycle/row, enables the DVE 4x mode, and halves HBM
traffic.  user_behavior is pre-converted to bf16 on the host.

On-chip layout ("t-parity packed transposed"): a core's tokens map to
columns n = tp*128 + b (tp = t//2), partitions hold (par, channel) with
par = t % 2.  The packed transpose of user_behavior is produced by
DMA-transpose straight from DRAM (one [128,128] xbar transpose per
t-pair) — no PE transposes or PSUM round-trip.  Matmuls use
block-diagonal weights blockdiag(W, W) so one K=128 matmul serves both
parity halves per 512-column chunk.

Algebra: att_in @ W1 decomposes as q@(W1a+W1c) + ub@(W1b-W1c) + (q*ub)@W1d.
The per-(b,c) term qW = q@Wq + b1 is accumulated into layer-1 PSUM via a
third matmul against a constant selector matrix (I_128 tiled 4x), so the
PSUM->SBUF copy is a plain copy.
"""

import os
import sys
from contextlib import ExitStack

import numpy as np

sys.path.insert(0, "/opt/trn_rl_repo")

import concourse.bacc as bacc
import concourse.bass as bass
import concourse.bass_isa as bass_isa
import concourse.library_config as library_config
import concourse.tile as tile
from concourse import mybir
from concourse.bass_utils import run_bass_kernel_spmd

F32 = mybir.dt.float32
BF16 = mybir.dt.bfloat16
I32 = mybir.dt.int32

N_CORES = 8
B, T, E = 1024, 200, 64
H1, H2 = 64, 16
Bs = B // N_CORES          # 128 batch rows per core
TP = T // 2                # 100 t-pairs
NCOL = TP * Bs             # 12800 packed columns per core
NCHUNK = NCOL // 512       # 25 column chunks of 512
EPS = 1e-8

# Module-global: test.py reads this after calling kernel() for trace info.
LAST_RESULTS = None


def _install_ntff_hook():
    """The agent image's antenv lacks axon_hooks, so trn_boot's NTFF hook
    registration silently degrades.  Recreate the registry module and set
    the ctypes-based hook so run_bass_kernel_spmd(trace=True) captures
    NTFF profiles."""
    import sys as _sys
    import types as _types

    try:
        from antenv.axon_hooks import get_axon_ntff_profile_hook  # noqa
        return
    except ImportError:
        pass
    try:
        from trn_agent_boot.trn_boot import _ntff_profile_via_ctypes
        hook = _ntff_profile_via_ctypes("/opt/axon/libaxon_pjrt.so")
        mod = _types.ModuleType("antenv.axon_hooks")
        _slot = [hook]
        mod.set_axon_ntff_profile_hook = lambda h: _slot.__setitem__(0, h)
        mod.get_axon_ntff_profile_hook = lambda: _slot[0]
        _sys.modules["antenv.axon_hooks"] = mod
        import antenv
        antenv.axon_hooks = mod
    except Exception as e:  # degrade silently — trace is optional
        print(f"ntff hook install failed: {e}")


def _ap(t, offset_elems, pattern):
    """Raw AP view over a tile AP (0-stride broadcasts, strided views).

    AP offsets are in elements; `pattern` dims are [elem_step, count] with
    dim0 = partition dim (pass through t.ap[0])."""
    return bass.AP(tensor=t.tensor, offset=t.offset + offset_elems, ap=pattern)


def _build(b3f, alphas_zero):
    nc = bacc.Bacc(
        "TRN2", target_bir_lowering=False, debug=False, num_devices=N_CORES
    )

    def din(name, shape, dt):
        return nc.dram_tensor(name, list(shape), dt, kind="ExternalInput").ap()

    ub_d = din("ub", [Bs, T, E], BF16)
    q_d = din("q", [Bs, E], BF16)
    lens_d = din("lens", [Bs, 1], I32)
    eye_d = din("eye", [32, 32], BF16)
    wublk_d = din("wublk", [128, 128], BF16)   # blockdiag(Wu, Wu)
    wpblk_d = din("wpblk", [128, 128], BF16)   # blockdiag(Wp, Wp)
    w2blk_d = din("w2blk", [128, 2 * H2], BF16)  # blockdiag(W2, W2)
    wq_d = din("wq", [E, H1], BF16)
    w3rep_d = din("w3rep", [128, 2 * H2], BF16)  # tile(W3, (128, 2))
    b1row_d = din("b1row", [128, H1], F32)     # b1 replicated down partitions
    b2dup_d = din("b2dup", [1, 2 * H2], BF16)
    sel_d = din("sel", [128, 512], BF16)       # I_128 tiled 4x
    iota_d = din("iota", [128, T], F32)
    if not alphas_zero:
        a1r_d = din("a1r", [128, 1], F32)
        am1r_d = din("am1r", [128, 1], F32)
        a2r_d = din("a2r", [2 * H2, 1], F32)
        am2r_d = din("am2r", [2 * H2, 1], F32)
    out_d = nc.dram_tensor("out", [Bs, E], F32, kind="ExternalOutput").ap()

    with tile.TileContext(nc) as tc, ExitStack() as ctx:
        # ---- pools ---------------------------------------------------------
        consts = ctx.enter_context(tc.tile_pool(name="consts", bufs=1))
        smalls = ctx.enter_context(tc.tile_pool(name="smalls", bufs=1))
        big = ctx.enter_context(tc.tile_pool(name="big", bufs=1))
        whp = ctx.enter_context(tc.tile_pool(name="whp", bufs=2))
        ph1 = ctx.enter_context(tc.tile_pool(name="ph1", bufs=3, space="PSUM"))
        ph2 = ctx.enter_context(tc.tile_pool(name="ph2", bufs=2, space="PSUM"))
        pt3 = ctx.enter_context(tc.tile_pool(name="pt3", bufs=2, space="PSUM"))

        # ---- packed DMA-transpose of ub, FIRST -----------------------------
        # The XPOSE descriptor struct only has one sync-wait slot, so every
        # other DMA is forced to schedule after the transposes (order-only
        # deps): cross-queue FIFO waits then land on copy-mode DMAs, which
        # have enough wait slots.
        ubT = big.tile([128, NCOL], BF16, tag="ubT")
        ubT3 = ubT.rearrange("p (k b) -> p k b", b=128)
        NTR = 5          # batched transposes, 20 t-pairs each
        TPB = TP // NTR
        tr_engines = [nc.sync, nc.scalar, nc.sync, nc.scalar, nc.sync]
        last_tr = None
        for g in range(NTR):
            # [128 b, 40 t, 64 e] -> out[(t%2,e), k, b] for 20 t-pairs k
            last_tr = tr_engines[g].dma_start_transpose(
                out=ubT3[:, TPB * g : TPB * (g + 1), :],
                in_=ub_d[:, 2 * TPB * g : 2 * TPB * (g + 1), :],
            )

        def dma(out, in_):
            inst = nc.gpsimd.dma_start(out=out, in_=in_)
            tile.add_dep_helper(inst.ins, last_tr.ins, sync=False,
                                reason="copy-mode DMA after xpose DMAs")
            return inst

        # ---- constants -----------------------------------------------------
        def cin(name, dram, shape, dt):
            t = consts.tile(shape, dt, name=name)
            dma(out=t, in_=dram)
            return t

        eye = cin("eye", eye_d, [32, 32], BF16)
        wublk = cin("wublk", wublk_d, [128, 128], BF16)
        wpblk = cin("wpblk", wpblk_d, [128, 128], BF16)
        w2blk = cin("w2blk", w2blk_d, [128, 2 * H2], BF16)
        wq = cin("wq", wq_d, [E, H1], BF16)
        w3rep = cin("w3rep", w3rep_d, [128, 2 * H2], BF16)
        b1row = cin("b1row", b1row_d, [128, H1], F32)
        b2dup = cin("b2dup", b2dup_d, [1, 2 * H2], BF16)
        sel = cin("sel", sel_d, [128, 512], BF16)
        iota = cin("iota", iota_d, [128, T], F32)
        lens_i = cin("lens_i", lens_d, [Bs, 1], I32)
        if not alphas_zero:
            a1r = cin("a1r", a1r_d, [128, 1], F32)
            am1r = cin("am1r", am1r_d, [128, 1], F32)
            a2r = cin("a2r", a2r_d, [2 * H2, 1], F32)
            am2r = cin("am2r", am2r_d, [2 * H2, 1], F32)

        lens_f = smalls.tile([Bs, 1], F32)
        nc.vector.tensor_copy(out=lens_f, in_=lens_i)
        eps_t = smalls.tile([128, 1], F32)
        nc.vector.memset(eps_t, EPS)

        # ---- q^T (DMA transpose), qW = q@Wq + b1 dup'd along free ----------
        qT = consts.tile([128, 128], BF16)
        dma(out=qT[0:E, :], in_=q_d.rearrange("a b -> b a"))
        dma(out=qT[E : 2 * E, :], in_=qT[0:E, :])

        pqw = ph1.tile([Bs, H1], F32, tag="pqw", bufs=1)
        nc.tensor.matmul(pqw, lhsT=qT[0:E, :], rhs=wq, start=True, stop=True)
        qWdup = consts.tile([128, 128], BF16)
        nc.vector.tensor_add(out=qWdup[:, 0:H1], in0=pqw, in1=b1row)
        nc.scalar.copy(out=qWdup[:, H1:128], in_=qWdup[:, 0:H1])

        # ---- big SBUF buffers (slots reused across phases via tags) --------
        qub = big.tile([128, NCOL], BF16, tag="qub")
        h1 = big.tile([128, NCOL], BF16, tag="h1")
        st1 = smalls.tile([128, NCHUNK, 6], F32)
        st2 = smalls.tile([32, NCHUNK, 6], F32)

        # ---- phase 1: qub -> L1 -> h1 --------------------------------------
        for i in range(NCHUNK):
            cols = slice(512 * i, 512 * (i + 1))
            # qub = ubT * q (q broadcast along the 4 tp's of the chunk)
            qbc = _ap(qT, 0, [qT.ap[0], [0, 4], [1, 128]])
            nc.vector.tensor_mul(out=qub[:, cols], in0=ubT[:, cols], in1=qbc)
            # L1: ub-term + qub-term + qW-via-selector, accumulated in PSUM
            p1t = ph1.tile([128, 512], F32, tag="h1p")
            nc.tensor.matmul(p1t, lhsT=wublk, rhs=ubT[:, cols],
                             start=True, stop=False)
            nc.tensor.matmul(p1t, lhsT=wpblk, rhs=qub[:, cols],
                             start=False, stop=False)
            nc.tensor.matmul(p1t, lhsT=qWdup, rhs=sel,
                             start=False, stop=True)
            if i % 2 == 0:
                nc.scalar.copy(out=h1[:, cols], in_=p1t)
            else:
                nc.vector.tensor_copy(out=h1[:, cols], in_=p1t)
            nc.vector.bn_stats(out=st1[:, i, :], in_=h1[:, cols])

        # ---- dice1 ---------------------------------------------------------
        def dice_stats(st, parts, key):
            mv = smalls.tile([parts, 2], F32, name=f"mv_{key}")
            nc.vector.bn_aggr(out=mv, in_=st)
            rs = smalls.tile([parts, 1], F32, name=f"rs_{key}")
            nc.scalar.activation(
                out=rs, in_=mv[:, 1:2], func=mybir.ActivationFunctionType.Sqrt,
                bias=eps_t[0:parts], scale=1.0,
            )
            nc.vector.reciprocal(out=rs, in_=rs)
            nb = smalls.tile([parts, 1], F32, name=f"nb_{key}")
            nc.vector.tensor_scalar(
                out=nb, in0=mv[:, 0:1], scalar1=rs, scalar2=-1.0,
                op0=mybir.AluOpType.mult, op1=mybir.AluOpType.mult,
            )
            return rs, nb

        rs1, nb1 = dice_stats(st1, 128, "d1")
        p1s = big.tile([128, NCOL], BF16, tag="qub")  # reuse qub slot
        for j in range(4):
            cols = slice(3200 * j, 3200 * (j + 1))
            nc.scalar.activation(
                out=p1s[:, cols], in_=h1[:, cols],
                func=mybir.ActivationFunctionType.Sigmoid, bias=nb1, scale=rs1,
            )
            if not alphas_zero:
                nc.vector.tensor_scalar(
                    out=p1s[:, cols], in0=p1s[:, cols], scalar1=am1r, scalar2=a1r,
                    op0=mybir.AluOpType.mult, op1=mybir.AluOpType.add,
                )
            nc.vector.tensor_mul(out=h1[:, cols], in0=h1[:, cols], in1=p1s[:, cols])

        # ---- L2: one block-diagonal matmul per chunk -> h2 [32, 12800] -----
        # b2 is accumulated in PSUM via a K=1 rank-1 matmul (b2 ⊗ ones) so
        # the PSUM->SBUF copy stays a plain table-free Copy.
        ones_row = smalls.tile([1, 512], BF16)
        nc.vector.memset(ones_row, 1.0)
        h2 = big.tile([32, NCOL], BF16, tag="ubT")  # reuse ubT slot
        for i in range(NCHUNK):
            cols = slice(512 * i, 512 * (i + 1))
            p2t = ph2.tile([32, 512], F32, tag="h2p")
            nc.tensor.matmul(p2t, lhsT=w2blk, rhs=h1[:, cols],
                             start=True, stop=False)
            nc.tensor.matmul(p2t, lhsT=b2dup, rhs=ones_row,
                             start=False, stop=True)
            if i % 2 == 0:
                nc.scalar.copy(out=h2[:, cols], in_=p2t)
            else:
                nc.vector.tensor_copy(out=h2[:, cols], in_=p2t)
            nc.vector.bn_stats(out=st2[:, i, :], in_=h2[:, cols])

        # ---- dice2 ---------------------------------------------------------
        rs2, nb2 = dice_stats(st2, 32, "d2")
        p2s = big.tile([32, NCOL], BF16, tag="qub")  # reuse qub slot again
        for j in range(4):
            cols = slice(3200 * j, 3200 * (j + 1))
            nc.scalar.activation(
                out=p2s[:, cols], in_=h2[:, cols],
                func=mybir.ActivationFunctionType.Sigmoid, bias=nb2, scale=rs2,
            )
            if not alphas_zero:
                nc.vector.tensor_scalar(
                    out=p2s[:, cols], in0=p2s[:, cols], scalar1=am2r, scalar2=a2r,
                    op0=mybir.AluOpType.mult, op1=mybir.AluOpType.add,
                )
            nc.vector.tensor_mul(out=h2[:, cols], in0=h2[:, cols], in1=p2s[:, cols])

        # ---- L3: transpose h2 back to batch-major, dot with W3 on DVE ------
        # Groups of 16 column-subblocks of 128: transpose [32,128] -> [128,32]
        sc = smalls.tile([Bs, T], F32)
        NG = (TP + 15) // 16  # 7 groups (6 full, last has 4 subblocks)
        for g in range(NG):
            nsub = min(16, TP - 16 * g)
            p3t = pt3.tile([128, 512], BF16, tag="t3")
            for s in range(nsub):
                k = 16 * g + s
                nc.tensor.matmul(
                    p3t[:, 32 * s : 32 * (s + 1)],
                    lhsT=h2[:, 128 * k : 128 * (k + 1)],
                    rhs=eye, is_transpose=True,
                )
            whj = whp.tile([128, 512], BF16, tag="wh2")
            w3b = _ap(w3rep, 0, [w3rep.ap[0], [0, nsub], [1, 32]])
            nc.vector.tensor_mul(
                out=whj[:, 0 : 32 * nsub], in0=p3t[:, 0 : 32 * nsub], in1=w3b
            )
            red_in = _ap(whj, 0, [whj.ap[0], [32, nsub], [16, 2], [1, 16]])
            red_out = _ap(sc, 32 * g, [sc.ap[0], [2, nsub], [1, 2]])
            nc.vector.tensor_reduce(
                out=red_out, in_=red_in, axis=mybir.AxisListType.X,
                op=mybir.AluOpType.add,
            )

        # ---- dice3 (per-core global stats via PE ones-matmul reduce) -------
        st3 = smalls.tile([128, 2], F32)
        nc.vector.reduce_sum(out=st3[:, 0:1], in_=sc, axis=mybir.AxisListType.X)
        sq_scr = smalls.tile([Bs, T], F32)
        nc.scalar.square(out=sq_scr, in_=sc)
        nc.vector.reduce_sum(out=st3[:, 1:2], in_=sq_scr,
                             axis=mybir.AxisListType.X)
        ones = smalls.tile([128, 1], F32)
        nc.vector.memset(ones, 1.0)
        ptot = ph1.tile([1, 2], F32, tag="pqw", bufs=1)
        nc.tensor.matmul(ptot, lhsT=ones, rhs=st3, start=True, stop=True)
        tot = smalls.tile([1, 2], F32)
        nc.vector.tensor_copy(out=tot, in_=ptot)
        ones_r = smalls.tile([1, 128], F32)
        nc.vector.memset(ones_r, 1.0)
        pbc = ph2.tile([128, 2], F32, tag="h2p")
        nc.tensor.matmul(pbc, lhsT=ones_r, rhs=tot, start=True, stop=True)
        st3r = smalls.tile([128, 2], F32)
        nc.vector.tensor_copy(out=st3r, in_=pbc)
        n_tok = float(Bs * T)
        m3 = smalls.tile([128, 1], F32)
        nc.vector.tensor_scalar(
            out=m3, in0=st3r[:, 0:1], scalar1=1.0 / n_tok, scalar2=None,
            op0=mybir.AluOpType.mult,
        )
        v3 = smalls.tile([128, 1], F32)
        nc.vector.tensor_scalar(
            out=v3, in0=st3r[:, 1:2], scalar1=1.0 / n_tok, scalar2=None,
            op0=mybir.AluOpType.mult,
        )
        msq = smalls.tile([128, 1], F32)
        nc.vector.tensor_mul(out=msq, in0=m3, in1=m3)
        nc.vector.tensor_sub(out=v3, in0=v3, in1=msq)
        rs3 = smalls.tile([128, 1], F32)
        nc.scalar.activation(
            out=rs3, in_=v3, func=mybir.ActivationFunctionType.Sqrt,
            bias=eps_t, scale=1.0,
        )
        nc.vector.reciprocal(out=rs3, in_=rs3)
        nb3 = smalls.tile([128, 1], F32)
        nc.vector.tensor_scalar(
            out=nb3, in0=m3, scalar1=b3f, scalar2=-1.0,
            op0=mybir.AluOpType.subtract, op1=mybir.AluOpType.mult,
        )
        nc.vector.tensor_mul(out=nb3, in0=nb3, in1=rs3)

        p3s = smalls.tile([Bs, T], F32)
        nc.scalar.activation(
            out=p3s, in_=sc, func=mybir.ActivationFunctionType.Sigmoid,
            bias=nb3, scale=rs3,
        )
        scd = smalls.tile([Bs, T], F32)
        if b3f != 0.0:
            scb = smalls.tile([Bs, T], F32)
            nc.scalar.add(out=scb, in_=sc, add=b3f)
            nc.vector.tensor_mul(out=scd, in0=p3s, in1=scb)
        else:
            nc.vector.tensor_mul(out=scd, in0=p3s, in1=sc)

        # ---- mask + softmax ------------------------------------------------
        pen = smalls.tile([Bs, T], F32)
        nc.vector.tensor_scalar(
            out=pen, in0=iota, scalar1=lens_f, scalar2=-1e30,
            op0=mybir.AluOpType.is_ge, op1=mybir.AluOpType.mult,
        )
        nc.vector.tensor_add(out=scd, in0=scd, in1=pen)
        nmx = smalls.tile([Bs, 1], F32)
        nc.vector.tensor_reduce(
            out=nmx, in_=scd, axis=mybir.AxisListType.X,
            op=mybir.AluOpType.max, negate=True,
        )
        wsm = smalls.tile([Bs, T], BF16)
        nc.scalar.activation(
            out=wsm, in_=scd, func=mybir.ActivationFunctionType.Exp,
            bias=nmx, scale=1.0,
        )
        ssum = smalls.tile([Bs, 1], F32)
        nc.vector.reduce_sum(out=ssum, in_=wsm, axis=mybir.AxisListType.X)
        rcp = smalls.tile([Bs, 1], F32)
        nc.vector.reciprocal(out=rcp, in_=ssum)

        # ---- weighted pooling over user_behavior ---------------------------
        # wub = ub * w (bf16, 4x DVE mode), then a contiguous binary
        # reduction tree over t (200 -> 100 -> 50 -> 25 -> strided finish).
        ub2 = big.tile([Bs, T, E], BF16, tag="ubT")  # reuse slot again
        for c in range(4):
            dma(out=ub2[:, 50 * c : 50 * (c + 1), :],
                in_=ub_d[:, 50 * c : 50 * (c + 1), :])
        wub = big.tile([Bs, T, E], BF16, tag="h1x")
        for c in range(4):
            tsl = slice(50 * c, 50 * (c + 1))
            wbc = _ap(wsm, 50 * c, [wsm.ap[0], [1, 50], [0, E]])
            nc.vector.tensor_mul(out=wub[:, tsl, :], in0=ub2[:, tsl, :], in1=wbc)
        tree = big.tile([Bs, 100, E], F32, tag="tree")
        nc.vector.tensor_add(out=tree, in0=wub[:, 0:100, :], in1=wub[:, 100:200, :])
        nc.vector.tensor_add(out=tree[:, 0:50, :], in0=tree[:, 0:50, :],
                             in1=tree[:, 50:100, :])
        nc.vector.tensor_add(out=tree[:, 0:25, :], in0=tree[:, 0:25, :],
                             in1=tree[:, 25:50, :])
        total = smalls.tile([Bs, E], F32)
        red_in = _ap(tree, 0, [tree.ap[0], [1, E], [E, 25]])
        nc.vector.tensor_reduce(
            out=total, in_=red_in, axis=mybir.AxisListType.X,
            op=mybir.AluOpType.add,
        )
        outt = smalls.tile([Bs, E], F32)
        nc.vector.tensor_scalar_mul(out=outt, in0=total, scalar1=rcp)
        dma(out=out_d, in_=outt)

    nc.compile()
    return nc


def _host_prep(query_ad, user_behavior, user_behavior_length,
               W1, b1, a1, W2, b2, a2, W3, b3, a3):
    import ml_dtypes
    f32 = np.float32
    bf16 = ml_dtypes.bfloat16
    q = np.ascontiguousarray(query_ad[:, 0, :]).astype(bf16)
    ub = np.ascontiguousarray(user_behavior).astype(bf16)
    lens = np.ascontiguousarray(user_behavior_length)[:, 0].astype(np.int32)
    W1 = np.asarray(W1, f32)
    Wq = W1[:E] + W1[2 * E : 3 * E]
    Wu = W1[E : 2 * E] - W1[2 * E : 3 * E]
    Wp = W1[3 * E :]
    W2 = np.asarray(W2, f32)
    W3 = np.asarray(W3, f32).reshape(-1)

    def blkdiag(w):
        k, m = w.shape
        out = np.zeros((2 * k, 2 * m), f32)
        out[:k, :m] = w
        out[k:, m:] = w
        return out.astype(bf16)

    consts = dict(
        eye=np.eye(32, dtype=f32).astype(bf16),
        wublk=blkdiag(Wu),
        wpblk=blkdiag(Wp),
        w2blk=blkdiag(W2),
        wq=np.ascontiguousarray(Wq).astype(bf16),
        w3rep=np.tile(W3, (128, 2)).astype(bf16),
        b1row=np.tile(np.asarray(b1, f32).reshape(1, H1), (128, 1)),
        b2dup=np.tile(np.asarray(b2, f32), 2).reshape(1, 2 * H2).astype(bf16),
        sel=np.tile(np.eye(128, dtype=f32), (1, 4)).astype(bf16),
        iota=np.broadcast_to(np.arange(T, dtype=f32), (128, T)).copy(),
    )
    b3f = float(np.asarray(b3, f32).reshape(-1)[0])
    a1 = np.asarray(a1, f32)
    a2 = np.asarray(a2, f32)
    a3f = float(np.asarray(a3, f32).reshape(-1)[0])
    alphas_zero = not a1.any() and not a2.any() and a3f == 0.0
    if not alphas_zero:
        if a3f != 0.0:
            raise NotImplementedError("nonzero alpha3 not supported")
        a1r = a1[(np.arange(128) % H1)].reshape(128, 1).astype(f32)
        a2r = np.tile(a2, 2).reshape(2 * H2, 1).astype(f32)
        consts.update(a1r=a1r, am1r=1.0 - a1r, a2r=a2r, am2r=1.0 - a2r)
    return q, ub, lens, consts, b3f, alphas_zero


def kernel(query_ad, user_behavior, user_behavior_length,
           W1, b1, a1, W2, b2, a2, W3, b3, a3):
    global LAST_RESULTS
    q, ub, lens, consts, b3f, alphas_zero = _host_prep(
        query_ad, user_behavior, user_behavior_length,
        W1, b1, a1, W2, b2, a2, W3, b3, a3)

    nc = _build(b3f, alphas_zero)

    in_maps = []
    for c in range(N_CORES):
        sl = slice(c * Bs, (c + 1) * Bs)
        m = dict(consts)
        m["ub"] = np.ascontiguousarray(ub[sl])
        m["q"] = np.ascontiguousarray(q[sl])
        m["lens"] = np.ascontiguousarray(lens[sl]).reshape(Bs, 1)
        in_maps.append(m)

    trace = bool(os.environ.get("BASS_TRACE"))
    if trace:
        _install_ntff_hook()
    res = run_bass_kernel_spmd(
        nc, in_maps, core_ids=list(range(N_CORES)), trace=trace,
    )
    LAST_RESULTS = res
    out = np.concatenate([res.results[c]["out"] for c in range(N_CORES)], axis=0)
    return out.astype(np.float32)
